# revision 7
# baseline (speedup 1.0000x reference)
"""Causal self-attention (nn.MultiheadAttention semantics) on 8 trn2 cores.

B=4, N=2048, C=1024, H=16, Dh=64; returns (out [B,N,C],
weights [B,N,N] = head-mean attention probs).

Sharding: core c = 2*b + half handles batch b, heads half*8..half*8+8.
Host gathers: out[b] = o[2b] + o[2b+1] + out_b (row-parallel out proj),
weights[b] = w[2b] + w[2b+1] (1/16 folded on device).

Per-core pipeline (all matmuls fp32r = full PE rate):
  proj:    qT/kT packed per head-pair [128=2x64d, N]; v_aug [128k, kc, h, 65]
           (65th col = ones -> attn matmul row 64 = softmax denominator).
  phase A: (attn, [k,q] orientation) scoresT chunks; ACT exp with
           per-partition pad bias; gpsimd affine_select on causal diagonal
           chunks; attnT accumulated over k chunks; normalize by 1/s.
  phase C: out projection of the finished q rows.
  phase B: (weights, [q,k] orientation) scores recomputed with q on
           partitions; pad via rank-1 bias-row matmul; ACT exp; one DVE
           scalar_tensor_tensor per tile: wsum = E * r16[q] + wsum where
           r16 = 1/(16*s), s transported from phase A via contract-1 matmul
           transposes. Causal diagonal cleaned once per q block.
"""
import numpy as np

B, N, C, H = 4, 2048, 1024, 16
DH = C // H          # 64
P = 128
NCORES = 8
HPC = 8              # heads per core
NPAIRS = HPC // 2    # 4
CC = C // P          # 8 contract chunks for projections
QT = 512             # q tile
NQT = N // QT        # 4

_cache = {}


def _build(VLM, VLSAFE):
    import concourse.tile as tile
    from concourse import bacc, mybir

    f32 = mybir.dt.float32
    f32r = mybir.dt.float32r
    AF = mybir.ActivationFunctionType
    OP = mybir.AluOpType
    KC = VLM // P                 # k chunks

    nc = bacc.Bacc(None, target_bir_lowering=False)

    t_xT = nc.dram_tensor("xT", [CC, P, N], f32r, kind="ExternalInput")
    t_wq = nc.dram_tensor("wq", [NPAIRS, CC, P, P], f32r, kind="ExternalInput")
    t_wk = nc.dram_tensor("wk", [NPAIRS, CC, P, P], f32r, kind="ExternalInput")
    t_wv = nc.dram_tensor("wv", [CC, P, HPC * DH], f32r, kind="ExternalInput")
    t_wo = nc.dram_tensor("wo", [NPAIRS, P, C], f32r, kind="ExternalInput")
    t_bq = nc.dram_tensor("bq", [P, NPAIRS], f32, kind="ExternalInput")
    t_bk = nc.dram_tensor("bk", [P, NPAIRS], f32, kind="ExternalInput")
    t_bv = nc.dram_tensor("bv", [1, HPC * DH], f32, kind="ExternalInput")
    t_kbp = nc.dram_tensor("kbp", [P, KC], f32, kind="ExternalInput")
    t_kbr = nc.dram_tensor("kbr", [1, VLM], f32, kind="ExternalInput")

    t_wout = nc.dram_tensor("w_out", [N, N], f32, kind="ExternalOutput")
    t_oout = nc.dram_tensor("o_out", [N, C], f32, kind="ExternalOutput")

    with tile.TileContext(nc) as tc:
        with tc.tile_pool(name="consts", bufs=1) as consts, \
             tc.tile_pool(name="qkv", bufs=1) as qkv, \
             tc.tile_pool(name="psA", bufs=2, space="PSUM") as psA, \
             tc.tile_pool(name="psAtt", bufs=2, space="PSUM") as psAtt, \
             tc.tile_pool(name="psB", bufs=2, space="PSUM") as psB:

            # ---- constants that live for the whole kernel ----
            wo_sb = consts.tile([P, NPAIRS, C], f32r, tag="wo")
            nc.sync.dma_start(wo_sb, t_wo[:, :, :].rearrange("j p m -> p j m"))
            bq_sb = consts.tile([P, NPAIRS], f32, tag="bq")
            nc.sync.dma_start(bq_sb, t_bq[:, :])
            bk_sb = consts.tile([P, NPAIRS], f32, tag="bk")
            nc.sync.dma_start(bk_sb, t_bk[:, :])
            bv_sb = consts.tile([P, HPC * DH], f32, tag="bv")
            nc.gpsimd.dma_start(bv_sb,
                                t_bv[0:1, :].to_broadcast((P, HPC * DH)))
            kbp_sb = consts.tile([P, KC], f32, tag="kbp")
            nc.sync.dma_start(kbp_sb, t_kbp[:, :])
            kbr_sb = consts.tile([1, VLM], f32, tag="kbr")
            nc.sync.dma_start(kbr_sb, t_kbr[:, :])
            ones1 = consts.tile([1, P], f32, tag="ones1")
            nc.vector.memset(ones1, 1.0)
            ones_col = consts.tile([P, 1], f32, tag="ones_col")
            nc.vector.memset(ones_col, 1.0)

            # ---- persistent q/k/v ----
            qT = [qkv.tile([P, N], f32r, tag=f"qT{j}", name=f"qT{j}") for j in range(NPAIRS)]
            kT = [qkv.tile([P, VLM], f32r, tag=f"kT{j}", name=f"kT{j}") for j in range(NPAIRS)]
            v_aug = qkv.tile([P, KC, HPC, DH + 1], f32r, tag="vaug")
            nc.vector.tensor_copy(
                v_aug[:, :, :, DH:DH + 1].rearrange("p a b c -> p (a b c)"),
                ones_col[:, 0:1].to_broadcast((P, KC * HPC)))

            # ---- projection (weights + x tiles in a scoped pool) ----
            with tc.tile_pool(name="projp", bufs=1) as projp, \
                 tc.tile_pool(name="xload", bufs=2) as xload:
                wq_sb = projp.tile([P, NPAIRS, CC, P], f32r, tag="wq")
                nc.sync.dma_start(wq_sb,
                                  t_wq[:, :, :, :].rearrange("j c p m -> p j c m"))
                wk_sb = projp.tile([P, NPAIRS, CC, P], f32r, tag="wk")
                nc.sync.dma_start(wk_sb,
                                  t_wk[:, :, :, :].rearrange("j c p m -> p j c m"))
                wv_sb = projp.tile([P, CC, HPC * DH], f32r, tag="wv")
                nc.sync.dma_start(wv_sb, t_wv[:, :, :].rearrange("c p m -> p c m"))

                for nt in range(NQT):
                    xt = xload.tile([P, CC, QT], f32r, tag="xt")
                    nc.sync.dma_start(
                        xt,
                        t_xT[:, :, nt * QT:(nt + 1) * QT].rearrange("c p n -> p c n"))
                    kw = min(QT, VLM - nt * QT)
                    for j in range(NPAIRS):
                        ps = psA.tile([P, 2, QT], f32, tag="psA")
                        for cc in range(CC):
                            nc.tensor.matmul(ps[:, 0, :], wq_sb[:, j, cc, :],
                                             xt[:, cc, :], start=(cc == 0),
                                             stop=(cc == CC - 1))
                        nc.vector.tensor_scalar_add(
                            qT[j][:, nt * QT:(nt + 1) * QT], ps[:, 0, :],
                            bq_sb[:, j:j + 1])
                        if kw > 0:
                            psk = psA.tile([P, 2, QT], f32, tag="psA")
                            for cc in range(CC):
                                nc.tensor.matmul(psk[:, 0, :kw],
                                                 wk_sb[:, j, cc, :],
                                                 xt[:, cc, :kw], start=(cc == 0),
                                                 stop=(cc == CC - 1))
                            nc.vector.tensor_scalar_add(
                                kT[j][:, nt * QT:nt * QT + kw], psk[:, 0, :kw],
                                bk_sb[:, j:j + 1])
                    if kw > 0:
                        for nb in range((kw + P - 1) // P):
                            kc = nt * 4 + nb
                            psv = psA.tile([P, 2, QT], f32, tag="psA")
                            for cc in range(CC):
                                nc.tensor.matmul(
                                    psv[:, 0, :HPC * DH],
                                    xt[:, cc, nb * P:(nb + 1) * P],
                                    wv_sb[:, cc, :], start=(cc == 0),
                                    stop=(cc == CC - 1))
                            nc.vector.tensor_tensor(
                                v_aug[:, kc, :, 0:DH],
                                psv[:, 0, :HPC * DH].rearrange(
                                    "p (h d) -> p h d", h=HPC),
                                bv_sb[:, :].rearrange("p (h d) -> p h d", h=HPC),
                                OP.add)

            # ---- attention + weights, pools for the main phases ----
            with tc.tile_pool(name="drp", bufs=4, space="DRAM") as drp, \
                 tc.tile_pool(name="expp", bufs=3) as expp, \
                 tc.tile_pool(name="ebp", bufs=3) as ebp, \
                 tc.tile_pool(name="attp", bufs=2) as attp, \
                 tc.tile_pool(name="wsump", bufs=2) as wsump, \
                 tc.tile_pool(name="sppool", bufs=8) as sppool, \
                 tc.tile_pool(name="miscp", bufs=4) as miscp, \
                 tc.tile_pool(name="outp", bufs=2) as outp:

                for qt in range(NQT):
                    att = attp.tile([P, NPAIRS, QT], f32r, tag="att")
                    s_sb = [None] * HPC
                    KCq = min(4 * (qt + 1), KC)
                    NGA = (KCq + 1) // 2
                    # ---- phase A ----
                    for j in range(NPAIRS):
                        for half in range(2):
                            h = 2 * j + half
                            base = 64 * half
                            pA = psAtt.tile([P, QT], f32, tag="pAtt")
                            for g in range(NGA):
                                glen = min(2, KCq - 2 * g)
                                psc = psA.tile([P, 2, QT], f32, tag="psA")
                                for i in range(glen):
                                    kc = 2 * g + i
                                    nc.tensor.matmul(
                                        psc[:, i, :],
                                        kT[j][base:base + 64, kc * P:(kc + 1) * P],
                                        qT[j][base:base + 64,
                                              qt * QT:(qt + 1) * QT],
                                        start=True, stop=True,
                                        tile_position=(base, 0))
                                eT = expp.tile([P, 2, QT], f32r, tag="eT")
                                both_valid = (2 * g + glen) * P <= VLSAFE
                                if glen == 2 and both_valid:
                                    nc.scalar.activation(
                                        eT[:, :, :].rearrange("p a b -> p (a b)"),
                                        psc[:, :, :].rearrange("p a b -> p (a b)"),
                                        AF.Exp, scale=0.125)
                                else:
                                    for i in range(glen):
                                        kc = 2 * g + i
                                        nc.scalar.activation(
                                            eT[:, i, :], psc[:, i, :], AF.Exp,
                                            bias=kbp_sb[:, kc:kc + 1],
                                            scale=0.125)
                                for i in range(glen):
                                    kc = 2 * g + i
                                    if kc >= 4 * qt:  # causal diagonal chunk
                                        nc.gpsimd.affine_select(
                                            eT[:, i, :], eT[:, i, :],
                                            pattern=[[1, QT]],
                                            compare_op=OP.is_ge, fill=0.0,
                                            base=qt * QT - kc * P,
                                            channel_multiplier=-1)
                                for i in range(glen):
                                    kc = 2 * g + i
                                    nc.tensor.matmul(
                                        pA[0:DH + 1, :], v_aug[:, kc, h, :],
                                        eT[:, i, :], start=(kc == 0),
                                        stop=(kc == KCq - 1))
                            ssb = sppool.tile([1, QT], f32, tag="ssb")
                            nc.vector.tensor_copy(ssb, pA[DH:DH + 1, :])
                            s_sb[h] = ssb
                            rec = miscp.tile([1, QT], f32, tag="rec")
                            nc.vector.reciprocal(rec, ssb)
                            rdram = drp.tile([1, QT], f32, tag="rdram")
                            nc.sync.dma_start(rdram, rec)
                            rec64 = miscp.tile([64, QT], f32, tag="rec64")
                            nc.gpsimd.dma_start(
                                rec64, rdram[0:1, :].to_broadcast((64, QT)))
                            nc.vector.tensor_tensor(
                                att[base:base + 64, j, :], pA[0:DH, :],
                                rec64, OP.mult)

                    # ---- phase C: out projection for these q rows ----
                    for nb in range(4):
                        oc = outp.tile([P, C], f32, tag="oc")
                        for oo in range(2):
                            pso = psAtt.tile([P, QT], f32, tag="pAtt")
                            for jj in range(NPAIRS):
                                nc.tensor.matmul(
                                    pso, att[:, jj, nb * P:(nb + 1) * P],
                                    wo_sb[:, jj, oo * QT:(oo + 1) * QT],
                                    start=(jj == 0), stop=(jj == NPAIRS - 1))
                            nc.scalar.activation(oc[:, oo * QT:(oo + 1) * QT],
                                                 pso, AF.Copy)
                        nc.sync.dma_start(
                            t_oout[(qt * 4 + nb) * P:(qt * 4 + nb + 1) * P, :],
                            oc)

                    # ---- transport s -> r16 = 1/(16 s) per partition ----
                    pT = psAtt.tile([P, QT], f32, tag="pAtt")
                    for h in range(HPC):
                        for qr in range(4):
                            nc.tensor.matmul(
                                pT[:, h * 4 + qr:h * 4 + qr + 1],
                                s_sb[h][0:1, qr * P:(qr + 1) * P],
                                ones1[0:1, 0:1], start=True, stop=True)
                    t16 = miscp.tile([P, HPC * 4], f32, tag="t16")
                    nc.vector.tensor_scalar_mul(t16, pT[:, 0:HPC * 4], 16.0)
                    r16 = miscp.tile([P, HPC * 4], f32, tag="r16")
                    nc.vector.reciprocal(r16, t16)

                    # ---- phase B: weights rows for q blocks of this tile ----
                    for qr in range(4):
                        qb = qt * 4 + qr
                        KW = min((qb + 1) * P, VLM)
                        NG = (KW + QT - 1) // QT
                        wsum = wsump.tile([P, VLM], f32, tag="wsum")
                        for j in range(NPAIRS):
                            pb0 = psB.tile([P, QT], f32, tag="psB")
                            pb1 = psB.tile([P, QT], f32, tag="psB")
                            pb = (pb0, pb1)
                            for g in range(NG):
                                gw = min(QT, KW - g * QT)
                                need_bias = g * QT + gw > VLSAFE
                                for half in range(2):
                                    base = 64 * half
                                    nc.tensor.matmul(
                                        pb[half][:, :gw],
                                        qT[j][base:base + 64,
                                              qb * P:(qb + 1) * P],
                                        kT[j][base:base + 64,
                                              g * QT:g * QT + gw],
                                        start=True, stop=not need_bias,
                                        tile_position=(base, 0))
                                    if need_bias:
                                        nc.tensor.matmul(
                                            pb[half][:, :gw], ones1[0:1, :],
                                            kbr_sb[0:1, g * QT:g * QT + gw],
                                            start=False, stop=True)
                                for half in range(2):
                                    h = 2 * j + half
                                    E = ebp.tile([P, QT], f32r, tag="EB")
                                    nc.scalar.activation(E[:, :gw],
                                                         pb[half][:, :gw],
                                                         AF.Exp, scale=0.125)
                                    rslice = r16[:, h * 4 + qr:h * 4 + qr + 1]
                                    if h == 0:
                                        nc.vector.tensor_scalar_mul(
                                            wsum[:, g * QT:g * QT + gw],
                                            E[:, :gw], rslice)
                                    else:
                                        nc.vector.scalar_tensor_tensor(
                                            wsum[:, g * QT:g * QT + gw],
                                            E[:, :gw], rslice,
                                            wsum[:, g * QT:g * QT + gw],
                                            OP.mult, OP.add)
                        if qb * P < VLM:
                            dw = KW - qb * P
                            nc.gpsimd.affine_select(
                                wsum[:, qb * P:KW], wsum[:, qb * P:KW],
                                pattern=[[-1, dw]], compare_op=OP.is_ge,
                                fill=0.0, base=0, channel_multiplier=1)
                        nc.sync.dma_start(t_wout[qb * P:(qb + 1) * P, 0:KW],
                                          wsum[:, :KW])

    nc.finalize()
    return nc


def _get_program(VLM, VLSAFE):
    key = (VLM, VLSAFE)
    if key not in _cache:
        _cache[key] = _build(VLM, VLSAFE)
    return _cache[key]


def kernel(x, key_padding_mask, in_proj_w, in_proj_b, out_w, out_b):
    from concourse.bass_utils import run_bass_kernel_spmd

    x = np.asarray(x, dtype=np.float32)
    kpm = np.asarray(key_padding_mask).astype(bool)
    in_proj_w = np.asarray(in_proj_w, dtype=np.float32)
    in_proj_b = np.asarray(in_proj_b, dtype=np.float32)
    out_w = np.asarray(out_w, dtype=np.float32)
    out_b = np.asarray(out_b, dtype=np.float32)

    vl = (~kpm).sum(axis=1).astype(int)   # valid key count per batch
    vl = np.maximum(vl, 1)
    VLM = int(min(N, ((vl.max() + P - 1) // P) * P))
    VLSAFE = int(vl.min())
    KC = VLM // P

    nc = _get_program(VLM, VLSAFE)

    Wq, Wk, Wv = in_proj_w[0:C], in_proj_w[C:2 * C], in_proj_w[2 * C:3 * C]
    bqf, bkf, bvf = (in_proj_b[0:C], in_proj_b[C:2 * C], in_proj_b[2 * C:3 * C])

    in_maps = []
    for c in range(NCORES):
        b, half = divmod(c, 2)
        hset = list(range(half * HPC, (half + 1) * HPC))
        rows = np.concatenate([np.arange(h * DH, (h + 1) * DH) for h in hset])

        def pack_pairs(W):
            out = np.empty((NPAIRS, CC, P, P), np.float32)
            for j in range(NPAIRS):
                h0, h1 = hset[2 * j], hset[2 * j + 1]
                blk = np.concatenate([W[h0 * DH:(h0 + 1) * DH],
                                      W[h1 * DH:(h1 + 1) * DH]], 0)
                out[j] = blk.T.reshape(CC, P, P)
            return out

        def pack_bias(bvec):
            out = np.empty((P, NPAIRS), np.float32)
            for j in range(NPAIRS):
                h0, h1 = hset[2 * j], hset[2 * j + 1]
                out[:, j] = np.concatenate([bvec[h0 * DH:(h0 + 1) * DH],
                                            bvec[h1 * DH:(h1 + 1) * DH]])
            return out

        kb = np.where(np.arange(VLM) < vl[b], 0.0, -1e30).astype(np.float32)
        in_maps.append({
            "xT": np.ascontiguousarray(x[b].T).reshape(CC, P, N),
            "wq": pack_pairs(Wq),
            "wk": pack_pairs(Wk),
            "wv": np.ascontiguousarray(Wv[rows].T).reshape(CC, P, HPC * DH),
            "wo": np.ascontiguousarray(out_w[:, rows].T).reshape(NPAIRS, P, C),
            "bq": pack_bias(bqf),
            "bk": pack_bias(bkf),
            "bv": bvf[rows].reshape(1, HPC * DH).astype(np.float32),
            "kbp": np.ascontiguousarray(kb.reshape(KC, P).T),
            "kbr": kb.reshape(1, VLM),
        })

    import os
    trace = bool(os.environ.get("KERNEL_TRACE"))
    res = run_bass_kernel_spmd(nc, in_maps, core_ids=list(range(NCORES)),
                               trace=trace)
    global _last_results
    _last_results = res
    results = res.results

    out = np.empty((B, N, C), np.float32)
    weights = np.empty((B, N, N), np.float32)
    for b in range(B):
        out[b] = results[2 * b]["o_out"] + results[2 * b + 1]["o_out"] + out_b
        weights[b] = results[2 * b]["w_out"] + results[2 * b + 1]["w_out"]
    return out, weights
